# revision 1
# baseline (speedup 1.0000x reference)
"""BiLSTM (2-layer, bidirectional) encoder + attention pooling on 8 Trainium2 cores.

Topology (one SPMD program, roles selected by partition id):
  pid 0: layer-0 forward LSTM  (full batch 128)      -> streams hT chunks via AG
  pid 1: layer-1 forward LSTM  (consumes pid0 chunks) -> streams h chunks via AG
  pid 2: layer-0 backward LSTM (host-reversed x)
  pid 3: layer-1 backward LSTM
  pid 4: attention core, f-side output (consumes pid1+pid3 streams)
  pid 5: attention core, b-side output
  pid 6,7: duplicates of 4,5 (ignored)

All transcendentals are Sigmoid (tanh(x) = 2*sig(2x)-1 with the 2x folded into
weights; exp(z) = sig(z)/(1-sig(z))), so the ACT table never reloads.
Matmuls run in float32r. Layer handoff: one 8-wide AllGather per 32-step chunk,
pipelined with lag 2; attention consumes chunks with lag 3 and accumulates the
softmax numerator middle-out with a fixed exp-shift M = sum(max(Wa2', 0)).
"""
import numpy as np

B, S_FULL, D, H, A = 128, 512, 256, 256, 64
CH = 32

_BUILD_CACHE = {}
_last_in_maps = None


def _build(S):
    import concourse.bass as bass
    from concourse import bacc
    import concourse.mybir as mybir
    from concourse.tile import TileContext
    from concourse.masks import make_identity

    F32 = mybir.dt.float32
    F32R = mybir.dt.float32r
    BF16 = mybir.dt.bfloat16
    AF = mybir.ActivationFunctionType
    OP = mybir.AluOpType

    NCH = S // CH
    LAG = 2          # L1 consumes chunk j at tick j+2
    ALAG = 3         # ATT consumes L1-chunk j at tick j+3
    NT = NCH + ALAG  # total ticks
    NAG = NCH + LAG  # AGs (ticks 0..NAG-1)
    MID = S // 2
    NW = NCH // 2    # windows

    nc = bacc.Bacc("TRN2", target_bir_lowering=False, debug=False, num_devices=8)

    xs_d = nc.dram_tensor("xs", [S, 2, 128, B], F32, kind="ExternalInput")
    wih_d = nc.dram_tensor("wih", [128, 2, 2, 512], F32, kind="ExternalInput")
    whh_d = nc.dram_tensor("whh", [128, 2, 2, 512], F32, kind="ExternalInput")
    bias_d = nc.dram_tensor("bias", [1, 1024], F32, kind="ExternalInput")
    wa1_d = nc.dram_tensor("wa1t", [128, 2, 2, A], F32, kind="ExternalInput")
    wa2_d = nc.dram_tensor("wa2r", [1, A], F32, kind="ExternalInput")
    mneg_d = nc.dram_tensor("mneg", [1, 1], F32, kind="ExternalInput")
    yout_d = nc.dram_tensor("yout", [B, H], F32, kind="ExternalOutput")

    agin = [nc.dram_tensor(f"agin{k}", [CH, 128, 256], F32) for k in range(NAG)]
    agbuf = [nc.dram_tensor(f"agbuf{k}", [8 * CH, 128, 256], F32) for k in range(NAG)]
    myin = [nc.dram_tensor(f"myin{j}", [CH, 128, 2, 128], F32) for j in range(NCH)]

    with TileContext(nc) as tc:
        with tc.tile_pool(name="wpool", bufs=1) as wp, \
             tc.tile_pool(name="xpool", bufs=6) as xp, \
             tc.tile_pool(name="gpool", bufs=3) as gp, \
             tc.tile_pool(name="apool", bufs=4) as ap_, \
             tc.tile_pool(name="psum", bufs=3, space="PSUM") as pp:

            pid = nc.partition_id()
            role = pid % 2 + (pid // 4) * 2

            # ---------------- prologue: constants and state ----------------
            wih = wp.tile([128, 2, 2, 512], BF16, tag="wih")
            nc.gpsimd.dma_start(out=wih[:], in_=wih_d.ap())
            whh = wp.tile([128, 2, 2, 512], F32R, tag="whh")
            nc.gpsimd.dma_start(out=whh[:], in_=whh_d.ap())
            bias_r = wp.tile([1, 1024], BF16, tag="bias_r")
            nc.gpsimd.dma_start(out=bias_r[:], in_=bias_d.ap())
            wa1 = wp.tile([128, 2, 2, A], F32R, tag="wa1")
            nc.gpsimd.dma_start(out=wa1[:], in_=wa1_d.ap())
            wa2bc = wp.tile([128, A], F32, tag="wa2bc")
            nc.sync.dma_start(out=wa2bc[:], in_=wa2_d.ap().partition_broadcast(128))
            mneg = wp.tile([128, 1], F32, tag="mneg")
            nc.sync.dma_start(out=mneg[:], in_=mneg_d.ap().partition_broadcast(128))
            ones_f = wp.tile([1, 128], F32, tag="ones_f")
            nc.gpsimd.memset(ones_f[:], 1.0)
            ones_r = wp.tile([1, 128], BF16, tag="ones_r")
            nc.vector.tensor_copy(out=ones_r[:], in_=ones_f[:])
            ident = wp.tile([128, 128], F32, tag="ident")
            make_identity(nc, ident[:])

            zini = wp.tile([128, 256], F32, tag="zini")
            nc.gpsimd.memset(zini[:], 0.0)
            hT_s = wp.tile([128, 2, 128], F32R, tag="hT_s")
            nc.vector.tensor_copy(out=hT_s[:].rearrange("p a b -> p (a b)"), in_=zini[:])
            c_s = wp.tile([128, 256], F32, tag="c_s")
            nc.vector.tensor_copy(out=c_s[:], in_=zini[:])

            # attention state
            a_s1 = wp.tile([128, S, A], BF16, tag="a_s1")
            a_s3 = wp.tile([128, S, A], BF16, tag="a_s3")
            acc = wp.tile([128, 256], F32, tag="acc")
            nc.vector.tensor_copy(out=acc[:], in_=zini[:])
            den = wp.tile([128, 1], F32, tag="den")
            nc.vector.tensor_copy(out=den[:], in_=zini[:, 0:1])
            e_sb = wp.tile([128, S], F32, tag="e_sb")

            def emit_partA(in_dma, sfx):
                """Input DMA + bias + input-projection matmuls (independent of h)."""
                xt = xp.tile([128, 2, 128], BF16, tag="xt", name=f"xt{sfx}")
                in_dma(xt)
                gb0 = pp.tile([128, 512], F32, tag="gb0", name=f"gb0{sfx}")
                gb1 = pp.tile([128, 512], F32, tag="gb1", name=f"gb1{sfx}")
                nc.tensor.matmul(gb0[:], ones_r[:], bias_r[:, 0:512],
                                 start=True, stop=False)
                nc.tensor.matmul(gb1[:], ones_r[:], bias_r[:, 512:1024],
                                 start=True, stop=False)
                for kc in range(2):
                    nc.tensor.matmul(gb0[:], xt[:, kc], wih[:, kc, 0],
                                     start=False, stop=False)
                    nc.tensor.matmul(gb1[:], xt[:, kc], wih[:, kc, 1],
                                     start=False, stop=False)
                return gb0, gb1

            def emit_partB(gb0, gb1, out_dma, sfx):
                """Recurrent matmuls + nonlinearity + state update."""
                for kc in range(2):
                    nc.tensor.matmul(gb0[:], hT_s[:, kc], whh[:, kc, 0],
                                     start=False, stop=(kc == 1))
                    nc.tensor.matmul(gb1[:], hT_s[:, kc], whh[:, kc, 1],
                                     start=False, stop=(kc == 1))
                sg = gp.tile([128, 1024], F32, tag="sg", name=f"sg{sfx}")
                nc.scalar.activation(sg[:, 0:512], gb0[:], AF.Sigmoid)
                nc.scalar.activation(sg[:, 512:1024], gb1[:], AF.Sigmoid)
                v = gp.tile([128, 256], F32, tag="v", name=f"v{sfx}")
                nc.vector.tensor_scalar(out=v[:], in0=sg[:, 768:1024],
                                        scalar1=2.0, scalar2=-1.0,
                                        op0=OP.mult, op1=OP.add)
                nc.vector.tensor_mul(out=v[:], in0=v[:], in1=sg[:, 0:256])
                nc.vector.tensor_mul(out=c_s[:], in0=c_s[:], in1=sg[:, 256:512])
                nc.vector.tensor_add(out=c_s[:], in0=c_s[:], in1=v[:])
                nc.scalar.activation(sg[:, 0:256], c_s[:], AF.Sigmoid, scale=2.0)
                w_t = gp.tile([128, 256], F32, tag="w_t", name=f"w{sfx}")
                nc.vector.tensor_scalar(out=w_t[:], in0=sg[:, 0:256],
                                        scalar1=2.0, scalar2=-1.0,
                                        op0=OP.mult, op1=OP.add)
                h_t = gp.tile([128, 256], F32, tag="h_t", name=f"h{sfx}")
                nc.gpsimd.tensor_mul(out=h_t[:], in0=sg[:, 512:768], in1=w_t[:])
                tp0 = pp.tile([128, 128], F32, bufs=1, tag="tp0", name=f"tp0{sfx}")
                tp1 = pp.tile([128, 128], F32, bufs=1, tag="tp1", name=f"tp1{sfx}")
                nc.tensor.transpose(tp0[:], h_t[:, 0:128], ident[:])
                nc.tensor.transpose(tp1[:], h_t[:, 128:256], ident[:])
                nc.vector.tensor_copy(out=hT_s[:, 0], in_=tp0[:])
                nc.vector.tensor_copy(out=hT_s[:, 1], in_=tp1[:])
                out_dma(h_t)

            PF = 2  # partA prefetch distance in steps

            def emit_chunk(tick, din_fn, dout_fn, pfx):
                pend = {}
                for t0 in range(min(PF, CH)):
                    pend[t0] = emit_partA(din_fn(t0), f"{pfx}{tick}_{t0}")
                for t0 in range(CH):
                    emit_partB(*pend.pop(t0), dout_fn(t0), f"{pfx}{tick}_{t0}")
                    ta = t0 + PF
                    if ta < CH:
                        pend[ta] = emit_partA(din_fn(ta), f"{pfx}{tick}_{ta}")

            for tick in range(NT):
                # ---------------- layer-0 producers ----------------
                if tick < NCH:
                    with tc.If(role == 0, name=f"L0t{tick}"):
                        def din_fn(t0, tick=tick):
                            t = tick * CH + t0
                            def din(xt):
                                nc.gpsimd.dma_start(
                                    out=xt[:], in_=xs_d.ap()[t].rearrange("a p b -> p a b"))
                            return din

                        def dout_fn(t0, tick=tick):
                            def dout(h_t):
                                nc.sync.dma_start(
                                    out=agin[tick].ap()[t0],
                                    in_=hT_s[:].rearrange("p a b -> p (a b)").bitcast(F32))
                            return dout

                        emit_chunk(tick, din_fn, dout_fn, "a")
                # ---------------- layer-1 consumers ----------------
                j = tick - LAG
                if 0 <= j < NCH:
                    with tc.If(role == 1, name=f"L1t{tick}"):
                        def din_fn(t0, j=j):
                            def din(xt):
                                nc.gpsimd.dma_start(out=xt[:], in_=myin[j].ap()[t0])
                            return din

                        def dout_fn(t0, tick=tick):
                            def dout(h_t):
                                nc.sync.dma_start(out=agin[tick].ap()[t0], in_=h_t[:])
                            return dout

                        emit_chunk(tick, din_fn, dout_fn, "b")
                # slot copies: agbuf[tick-1] slot (pid-1) -> myin[tick-1]
                jj = tick - 1
                if 0 <= jj < NCH:
                    for cpid, slot in ((1, 0), (3, 2)):
                        with tc.If(pid == cpid, name=f"cp{tick}_{cpid}"):
                            nc.sync.dma_start(
                                out=myin[jj].ap(),
                                in_=agbuf[jj].ap()[slot * CH:(slot + 1) * CH]
                                .rearrange("c p (a b) -> c p a b", a=2))
                # ---------------- attention arrival processing ----------------
                ja = tick - ALAG
                if 0 <= ja < NCH:
                    with tc.If(pid >= 4, name=f"ATarr{tick}"):
                        for t0 in range(CH):
                            t = ja * CH + t0
                            for slot, atile in ((1, a_s1), (3, a_s3)):
                                hr = ap_.tile([128, 256], F32, bufs=2, tag=f"hr{slot}",
                                              name=f"hr{slot}_{tick}_{t0}")
                                nc.sync.dma_start(
                                    out=hr[:], in_=agbuf[ja + LAG].ap()[slot * CH + t0])
                                tq0 = pp.tile([128, 128], F32, bufs=1, tag="tp0",
                                              name=f"tq0_{slot}_{tick}_{t0}")
                                tq1 = pp.tile([128, 128], F32, bufs=1, tag="tp1",
                                              name=f"tq1_{slot}_{tick}_{t0}")
                                nc.tensor.transpose(tq0[:], hr[:, 0:128], ident[:])
                                nc.tensor.transpose(tq1[:], hr[:, 128:256], ident[:])
                                hTr = ap_.tile([128, 2, 128], F32R, bufs=2, tag=f"hTr{slot}",
                                               name=f"hTr{slot}_{tick}_{t0}")
                                nc.vector.tensor_copy(out=hTr[:, 0], in_=tq0[:])
                                nc.vector.tensor_copy(out=hTr[:, 1], in_=tq1[:])
                                fb = 0 if slot == 1 else 1
                                aps = pp.tile([128, A], F32, tag="gb0" if slot == 1 else "gb1",
                                              name=f"aps{slot}_{tick}_{t0}")
                                nc.tensor.matmul(aps[:], hTr[:, 0], wa1[:, 0, fb],
                                                 start=True, stop=False)
                                nc.tensor.matmul(aps[:], hTr[:, 1], wa1[:, 1, fb],
                                                 start=False, stop=True)
                                nc.vector.tensor_copy(out=atile[:, t, :], in_=aps[:])
                # ---------------- attention windows ----------------
                w = tick - (NCH // 2 + ALAG)
                if 0 <= w < NW:
                    for rl, aloc, arem, slot_loc in ((2, a_s1, a_s3, 1), (3, a_s3, a_s1, 3)):
                        with tc.If(role == rl, name=f"ATw{tick}_{rl}"):
                            for bs in (MID - CH * (w + 1), MID + CH * w):
                                asum = ap_.tile([128, CH, A], F32, bufs=1, tag="asum",
                                                name=f"as{tick}_{rl}_{bs}")
                                nc.vector.tensor_add(
                                    out=asum[:], in0=aloc[:, bs:bs + CH, :],
                                    in1=arem[:, S - bs - CH:S - bs, :][:, ::-1, :])
                                nc.scalar.activation(asum[:], asum[:], AF.Sigmoid)
                                nc.vector.tensor_mul(
                                    out=asum[:], in0=asum[:],
                                    in1=wa2bc[:].unsqueeze(1).broadcast_to([128, CH, A]))
                                sco = ap_.tile([128, CH], F32, bufs=2, tag="sco",
                                               name=f"sc{tick}_{rl}_{bs}")
                                nc.vector.reduce_sum(out=sco[:], in_=asum[:],
                                                     axis=mybir.AxisListType.X)
                                nc.scalar.activation(sco[:], sco[:], AF.Sigmoid,
                                                     bias=mneg[:, 0:1])
                                dtile = ap_.tile([128, CH], F32, bufs=2, tag="dtile",
                                                 name=f"dt{tick}_{rl}_{bs}")
                                nc.vector.tensor_scalar(out=dtile[:], in0=sco[:],
                                                        scalar1=-1.0, scalar2=1.0,
                                                        op0=OP.mult, op1=OP.add)
                                nc.vector.reciprocal(out=dtile[:], in_=dtile[:])
                                nc.vector.tensor_mul(out=e_sb[:, bs:bs + CH],
                                                     in0=sco[:], in1=dtile[:])
                                nc.vector.reduce_sum(out=dtile[:, 0:1],
                                                     in_=e_sb[:, bs:bs + CH],
                                                     axis=mybir.AxisListType.X)
                                nc.vector.tensor_add(out=den[:], in0=den[:],
                                                     in1=dtile[:, 0:1])
                                for u in range(bs, bs + CH):
                                    hw_ = ap_.tile([128, 256], F32, bufs=3, tag="hw",
                                                   name=f"hw{tick}_{rl}_{u}")
                                    nc.sync.dma_start(
                                        out=hw_[:],
                                        in_=agbuf[u // CH + LAG].ap()[slot_loc * CH + u % CH])
                                    nc.vector.scalar_tensor_tensor(
                                        out=acc[:], in0=hw_[:], scalar=e_sb[:, u:u + 1],
                                        in1=acc[:], op0=OP.mult, op1=OP.add)
                # ---------------- finalize ----------------
                if tick == NT - 1:
                    with tc.If(pid >= 4, name="fin"):
                        rden = ap_.tile([128, 1], F32, bufs=1, tag="rden")
                        nc.vector.reciprocal(out=rden[:], in_=den[:])
                        yt = ap_.tile([128, 256], F32, bufs=1, tag="yt")
                        nc.vector.tensor_scalar_mul(yt[:], acc[:], rden[:, 0:1])
                        nc.sync.dma_start(out=yout_d.ap(), in_=yt[:])
                # ---------------- AllGather ----------------
                if tick < NAG:
                    nc.gpsimd.collective_compute(
                        "AllGather", mybir.AluOpType.bypass,
                        replica_groups=[list(range(8))],
                        ins=[agin[tick].ap()], outs=[agbuf[tick].ap()],
                    )

    nc.compile()
    return nc


def _prep_lstm_w(Wih, Whh, bih, bhh):
    def reorder(M):
        return np.concatenate([M[0:512], M[768:1024], 2.0 * M[512:768]], axis=0)

    wih_t = np.ascontiguousarray(reorder(Wih).T)   # [256, 1024]
    whh_t = np.ascontiguousarray(reorder(Whh).T)
    bias = reorder((bih + bhh).reshape(1024, 1)).reshape(1, 1024)

    def chunk(WT):
        return np.ascontiguousarray(
            WT.reshape(2, 128, 2, 512).transpose(1, 0, 2, 3)).astype(np.float32)

    return chunk(wih_t), chunk(whh_t), bias.astype(np.float32)


def kernel(**inputs):
    from concourse.bass_utils import run_bass_kernel_spmd

    x = np.asarray(inputs["x"], np.float32)
    Bv, S, Dv = x.shape
    if (S, "nc") not in _BUILD_CACHE:
        _BUILD_CACHE[(S, "nc")] = _build(S)
    nc = _BUILD_CACHE[(S, "nc")]

    xs_f = np.ascontiguousarray(x.transpose(1, 2, 0)).reshape(S, 2, 128, Bv)
    xs_b = np.ascontiguousarray(x[:, ::-1].transpose(1, 2, 0)).reshape(S, 2, 128, Bv)
    z_xs = np.zeros((S, 2, 128, Bv), np.float32)
    zw = np.zeros((128, 2, 2, 512), np.float32)
    zb = np.zeros((1, 1024), np.float32)

    wf0 = _prep_lstm_w(inputs["Wih_f0"], inputs["Whh_f0"], inputs["bih_f0"], inputs["bhh_f0"])
    wf1 = _prep_lstm_w(inputs["Wih_f1"], inputs["Whh_f1"], inputs["bih_f1"], inputs["bhh_f1"])
    wb0 = _prep_lstm_w(inputs["Wih_b0"], inputs["Whh_b0"], inputs["bih_b0"], inputs["bhh_b0"])
    wb1 = _prep_lstm_w(inputs["Wih_b1"], inputs["Whh_b1"], inputs["bih_b1"], inputs["bhh_b1"])

    wa1p = 2.0 * np.asarray(inputs["Wa1"], np.float32)        # [64, 512]
    wa1t = np.ascontiguousarray(
        wa1p.T.reshape(2, 2, 128, A).transpose(2, 1, 0, 3)).astype(np.float32)
    wa2p = (2.0 * np.asarray(inputs["Wa2"], np.float32)).reshape(1, A)
    mconst = np.float32(np.maximum(wa2p, 0.0).sum())
    mneg = np.full((1, 1), -mconst, np.float32)

    def imap(xs, w3, use_att=True):
        wih, whh, bias = w3
        return {"xs": xs, "wih": wih, "whh": whh, "bias": bias,
                "wa1t": wa1t, "wa2r": wa2p, "mneg": mneg}

    zero3 = (zw, zw, zb)
    in_maps = [
        imap(xs_f, wf0), imap(z_xs, wf1),
        imap(xs_b, wb0), imap(z_xs, wb1),
        imap(z_xs, zero3), imap(z_xs, zero3),
        imap(z_xs, zero3), imap(z_xs, zero3),
    ]
    global _last_in_maps
    _last_in_maps = in_maps
    res = run_bass_kernel_spmd(nc, in_maps, core_ids=list(range(8)))
    out = np.concatenate([res.results[4]["yout"], res.results[5]["yout"]], axis=1)
    return out.astype(np.float32)


# expose wa1t chunk helper constant for tests
def _wa1_layout_check():
    pass



# revision 14
# speedup vs baseline: 1.2792x; 1.2792x over previous
"""BiLSTM (2-layer, bidirectional) encoder + attention pooling on 8 Trainium2 cores.

Topology (one SPMD program, roles selected by partition id):
  pid 0: layer-0 forward LSTM  (full batch 128)          role 0
  pid 2: layer-0 backward LSTM (host-reversed x)         role 0
  pid 1: layer-1 forward LSTM + f-side attention + f-half output   role 1
  pid 3: layer-1 backward LSTM + b-side attention + b-half output  role 1
  pid 4-7: idle (participate in collectives with garbage pairs)

Communication per 32-step chunk (all bf16):
  AG1 groups [[0,1],[2,3],[4,5],[6,7]]: L0 hidden states (transposed layout)
      2MB wire -> L1 consumes with lag 2.
  AG2 groups [[1,3],[0,2],[4,6],[5,7]]: A=64 attention pre-projections
      0.5MB wire -> peer attention scores.

Gates are reordered host-side to [g|o|i|f] so bank0 of the PSUM gate tile
finishes first and tanh(g) can start early; tanh and sigmoid share one ACT
table set so there are no table reloads.  exp for the softmax uses the exact
identity exp(z) = sig(z)/(1-sig(z)) with a fixed shift M = sum|Wa2| so scores
can be consumed in any order; windows run middle-out as both directions'
scores become available.
"""
import numpy as np

B, D, H, A = 128, 256, 256, 64
CH = 32

_BUILD_CACHE = {}
_last_in_maps = None


def _build(S):
    import concourse.bass as bass
    from concourse import bacc
    import concourse.mybir as mybir
    from concourse.tile import TileContext
    from concourse.masks import make_identity

    F32 = mybir.dt.float32
    BF16 = mybir.dt.bfloat16
    AF = mybir.ActivationFunctionType
    OP = mybir.AluOpType

    NCH = S // CH
    LAG = 2                       # L1 consumes L0 chunk j at tick j+LAG
    NW = NCH // 2                 # middle-out window pairs
    WSTART = NCH // 2 + LAG + 2   # first window tick
    NT = WSTART + NW + 1          # + finalize tick
    MID = S // 2

    G1 = [[0, 1], [2, 3], [4, 5], [6, 7]]
    G2 = [[1, 3], [0, 2], [4, 6], [5, 7]]

    nc = bacc.Bacc("TRN2", target_bir_lowering=False, debug=False, num_devices=8)

    xs_d = nc.dram_tensor("xs", [S, 2, 128, B], BF16, kind="ExternalInput")
    wih_d = nc.dram_tensor("wih", [128, 2, 2, 512], BF16, kind="ExternalInput")
    whh_d = nc.dram_tensor("whh", [128, 2, 2, 512], BF16, kind="ExternalInput")
    bias_d = nc.dram_tensor("bias", [1, 1024], BF16, kind="ExternalInput")
    wa1_d = nc.dram_tensor("wa1t", [128, 2, A], BF16, kind="ExternalInput")
    wa2_d = nc.dram_tensor("wa2r", [1, A], BF16, kind="ExternalInput")
    mneg_d = nc.dram_tensor("mneg", [1, 1], F32, kind="ExternalInput")
    yout_d = nc.dram_tensor("yout", [B, H], F32, kind="ExternalOutput")

    agin1 = [nc.dram_tensor(f"agin1_{j}", [CH, 128, 256], BF16) for j in range(NCH)]
    agbuf1 = [nc.dram_tensor(f"agbuf1_{j}", [2 * CH, 128, 256], BF16)
              for j in range(NCH)]
    agin2 = [nc.dram_tensor(f"agin2_{j}", [128, CH, A], BF16) for j in range(NCH)]
    agbuf2 = [nc.dram_tensor(f"agbuf2_{j}", [2 * 128, CH, A], BF16)
              for j in range(NCH)]
    hist_d = nc.dram_tensor("hist", [NCH, 128, CH, 256], BF16)
    # per-pid slot pull lands here (ring of 2 window ticks, 2 blocks each)
    aremd = [nc.dram_tensor(f"aremd_{k}", [2, 128, CH, A], BF16) for k in range(2)]

    with TileContext(nc) as tc:
        with tc.tile_pool(name="wpool", bufs=1) as wp, \
             tc.tile_pool(name="xpool", bufs=2) as xp, \
             tc.tile_pool(name="gpool", bufs=2) as gp, \
             tc.tile_pool(name="apool", bufs=2) as ap_, \
             tc.tile_pool(name="psum", bufs=2, space="PSUM") as pp:

            pid = nc.partition_id()
            role = pid % 2 + (pid // 4) * 2

            # ---------------- prologue: constants and state ----------------
            wih = wp.tile([128, 2, 2, 512], BF16, tag="wih")
            nc.gpsimd.dma_start(out=wih[:], in_=wih_d.ap())
            whh = wp.tile([128, 2, 2, 512], BF16, tag="whh")
            nc.gpsimd.dma_start(out=whh[:], in_=whh_d.ap())
            bias_r = wp.tile([1, 1024], BF16, tag="bias_r")
            nc.gpsimd.dma_start(out=bias_r[:], in_=bias_d.ap())
            wa1 = wp.tile([128, 2, A], BF16, tag="wa1")
            nc.gpsimd.dma_start(out=wa1[:], in_=wa1_d.ap())
            wa2bc = wp.tile([128, A], BF16, tag="wa2bc")
            nc.sync.dma_start(out=wa2bc[:], in_=wa2_d.ap().partition_broadcast(128))
            wa2rep = wp.tile([128, CH, A], BF16, tag="wa2rep")
            for r in range(CH):
                nc.vector.tensor_copy(out=wa2rep[:, r], in_=wa2bc[:])
            mneg = wp.tile([128, 1], F32, tag="mneg")
            nc.sync.dma_start(out=mneg[:], in_=mneg_d.ap().partition_broadcast(128))
            ones_r = wp.tile([1, 128], BF16, tag="ones_r")
            nc.gpsimd.memset(ones_r[:], 1.0)
            ident = wp.tile([128, 128], F32, tag="ident")
            make_identity(nc, ident[:])
            identb = wp.tile([128, 128], BF16, tag="identb")
            nc.vector.tensor_copy(out=identb[:], in_=ident[:])

            zini = wp.tile([128, 256], F32, tag="zini")
            nc.gpsimd.memset(zini[:], 0.0)
            # persistent prev-step transposed h (per role; only one role runs
            # per core but tiles are statically allocated for both)
            hTp0 = wp.tile([128, 2, 128], BF16, tag="hTp0")
            nc.gpsimd.memset(hTp0[:], 0.0)
            hTp1 = wp.tile([128, 2, 128], BF16, tag="hTp1")
            nc.gpsimd.memset(hTp1[:], 0.0)
            hT_prev = {0: hTp0, 1: hTp1}
            c_s = wp.tile([128, 256], F32, tag="c_s")
            nc.vector.tensor_copy(out=c_s[:], in_=zini[:])

            # attention state
            acc_v = wp.tile([128, 256], F32, tag="acc_v")
            nc.vector.tensor_copy(out=acc_v[:], in_=zini[:])
            den = wp.tile([128, 1], F32, tag="den")
            nc.vector.tensor_copy(out=den[:], in_=zini[:, 0:1])

            def emit_lstm_chunk(cj, is_l1, sfx):
                """One CH-step chunk of LSTM; role0 ships hT, role1 also
                computes attention projections and h history."""
                xt = xp.tile([128, CH, 2, 128], BF16, tag="xtc", name=f"xtc{sfx}")
                if is_l1:
                    nc.sync.dma_start(
                        out=xt[:],
                        in_=agbuf1[cj].ap()[0:CH]
                        .rearrange("c p (a b) -> p c a b", a=2))
                else:
                    nc.sync.dma_start(
                        out=xt[:],
                        in_=xs_d.ap()[cj * CH:(cj + 1) * CH]
                        .rearrange("c a p b -> p c a b"))
                hTc = xp.tile([128, CH, 2, 128], BF16, tag="hTc", name=f"hTc{sfx}")
                if is_l1:
                    hc = xp.tile([128, CH, 256], BF16, tag="hc", name=f"hc{sfx}")
                    a_ch = ap_.tile([128, CH, A], BF16, tag="ach", name=f"ach{sfx}")

                pgb = [None, None]  # rotating python refs for gb tiles

                def emit_partA(t0):
                    gb = pp.tile([128, 1088], F32, tag="gb", name=f"gb{sfx}_{t0}")
                    # bank0 = [g|o], bank1 = [i|f]
                    nc.tensor.matmul(gb[:, 0:512], ones_r[:], bias_r[:, 0:512],
                                     start=True, stop=False)
                    for kc in range(2):
                        nc.tensor.matmul(gb[:, 0:512], xt[:, t0, kc], wih[:, kc, 0],
                                         start=False, stop=False)
                    nc.tensor.matmul(gb[:, 512:1024], ones_r[:], bias_r[:, 512:1024],
                                     start=True, stop=False)
                    for kc in range(2):
                        nc.tensor.matmul(gb[:, 512:1024], xt[:, t0, kc],
                                         wih[:, kc, 1], start=False, stop=False)
                    return gb

                def emit_partB(gb, t0):
                    if t0 == 0:
                        prevT = hT_prev[1 if is_l1 else 0][:]
                    else:
                        prevT = hTc[:, t0 - 1]
                    # bank0 ([g|o]) completes first so tanh(g) starts early
                    nc.tensor.matmul(gb[:, 0:512], prevT[:, 0], whh[:, 0, 0],
                                     start=False, stop=False)
                    nc.tensor.matmul(gb[:, 0:512], prevT[:, 1], whh[:, 1, 0],
                                     start=False, stop=True)
                    nc.tensor.matmul(gb[:, 512:1024], prevT[:, 0], whh[:, 0, 1],
                                     start=False, stop=False)
                    nc.tensor.matmul(gb[:, 512:1024], prevT[:, 1], whh[:, 1, 1],
                                     start=False, stop=True)
                    if is_l1 and t0 >= 1:
                        # attention projection of step t0-1 (same stationaries)
                        nc.tensor.matmul(gb[:, 1024:1088], prevT[:, 0], wa1[:, 0],
                                         start=True, stop=False)
                        nc.tensor.matmul(gb[:, 1024:1088], prevT[:, 1], wa1[:, 1],
                                         start=False, stop=True)
                    sg = gp.tile([128, 1024], F32, tag="sg", name=f"sg{sfx}_{t0}")
                    nc.scalar.activation(sg[:, 0:256], gb[:, 0:256], AF.Tanh)
                    nc.scalar.activation(sg[:, 256:1024], gb[:, 256:1024], AF.Sigmoid)
                    # layout: tanh_g | sig_o | sig_i | sig_f
                    v = gp.tile([128, 256], F32, tag="v", name=f"v{sfx}_{t0}")
                    nc.gpsimd.tensor_mul(out=v[:], in0=sg[:, 0:256],
                                         in1=sg[:, 512:768])
                    nc.vector.tensor_mul(out=c_s[:], in0=c_s[:], in1=sg[:, 768:1024])
                    nc.vector.tensor_add(out=c_s[:], in0=c_s[:], in1=v[:])
                    th = gp.tile([128, 256], F32, tag="th", name=f"th{sfx}_{t0}")
                    nc.scalar.activation(th[:], c_s[:], AF.Tanh)
                    if is_l1:
                        hdst = hc[:, t0]
                    else:
                        h0 = gp.tile([128, 256], BF16, tag="h0", name=f"h0{sfx}_{t0}")
                        hdst = h0[:]
                    nc.gpsimd.tensor_mul(out=hdst, in0=sg[:, 256:512], in1=th[:])
                    tp = pp.tile([128, 256], BF16, tag="tp", name=f"tp{sfx}_{t0}")
                    nc.tensor.transpose(tp[:, 0:128], hdst[:, 0:128], identb[:])
                    nc.tensor.transpose(tp[:, 128:256], hdst[:, 128:256], identb[:])
                    nc.vector.tensor_copy(
                        out=hTc[:, t0].rearrange("p a b -> p (a b)"), in_=tp[:])
                    if is_l1 and t0 >= 1:
                        nc.vector.tensor_copy(out=a_ch[:, t0 - 1],
                                              in_=gb[:, 1024:1088])

                pgb[0] = emit_partA(0)
                for t0 in range(CH):
                    if t0 + 1 < CH:
                        pgb[1] = emit_partA(t0 + 1)
                    emit_partB(pgb[0], t0)
                    pgb[0] = pgb[1]

                if is_l1:
                    # tail: attention projection of step CH-1
                    aps_t = pp.tile([128, 64], F32, tag="tp", name=f"apst{sfx}")
                    nc.tensor.matmul(aps_t[:], hTc[:, CH - 1, 0], wa1[:, 0],
                                     start=True, stop=False)
                    nc.tensor.matmul(aps_t[:], hTc[:, CH - 1, 1], wa1[:, 1],
                                     start=False, stop=True)
                    nc.vector.tensor_copy(out=a_ch[:, CH - 1], in_=aps_t[:])
                    nc.sync.dma_start(out=agin2[cj].ap(), in_=a_ch[:])
                    nc.sync.dma_start(out=hist_d.ap()[cj], in_=hc[:])
                else:
                    nc.sync.dma_start(
                        out=agin1[cj].ap().rearrange("c p f -> p c f"),
                        in_=hTc[:].rearrange("p c a b -> p c (a b)"))
                nc.vector.tensor_copy(
                    out=hT_prev[1 if is_l1 else 0][:].rearrange("p a b -> p (a b)"),
                    in_=hTc[:, CH - 1].rearrange("p a b -> p (a b)"))

            def emit_window(w, sfx):
                """Score + weighted-accumulate for the window-pair w
                (own-time blocks around MID). Slot-independent part."""
                for bi, bs in enumerate((MID - CH * (w + 1), MID + CH * w)):
                    cf = bs // CH
                    aown = ap_.tile([128, CH, A], BF16, tag="aown",
                                    name=f"aown{sfx}_{bi}")
                    nc.sync.dma_start(out=aown[:], in_=agin2[cf].ap())
                    arem = ap_.tile([128, CH, A], BF16, tag="arem",
                                    name=f"arem{sfx}_{bi}")
                    nc.sync.dma_start(out=arem[:], in_=aremd[w % 2].ap()[bi])
                    asum = ap_.tile([128, CH, A], BF16, tag="asum",
                                    name=f"asum{sfx}_{bi}")
                    nc.vector.tensor_add(out=asum[:], in0=aown[:],
                                         in1=arem[:][:, ::-1, :])
                    nc.scalar.activation(asum[:], asum[:], AF.Tanh)
                    nc.vector.tensor_mul(out=asum[:], in0=asum[:], in1=wa2rep[:])
                    sco = ap_.tile([128, CH], F32, tag="sco", name=f"sco{sfx}_{bi}")
                    nc.vector.tensor_reduce(out=sco[:], in_=asum[:],
                                            axis=mybir.AxisListType.X,
                                            op=OP.add)
                    nc.scalar.activation(sco[:], sco[:], AF.Sigmoid,
                                         bias=mneg[:, 0:1])
                    dtl = ap_.tile([128, CH], F32, tag="dtl", name=f"dtl{sfx}_{bi}")
                    nc.vector.tensor_scalar(out=dtl[:], in0=sco[:],
                                            scalar1=-1.0, scalar2=1.0,
                                            op0=OP.mult, op1=OP.add)
                    nc.vector.reciprocal(out=dtl[:], in_=dtl[:])
                    e_blk = ap_.tile([128, CH], F32, tag="e_blk",
                                     name=f"eb{sfx}_{bi}")
                    dinc = ap_.tile([128, 1], F32, tag="dinc",
                                    name=f"di{sfx}_{bi}")
                    nc.vector.scalar_tensor_tensor(
                        out=e_blk[:], in0=sco[:], scalar=1.0, in1=dtl[:],
                        op0=OP.mult, op1=OP.mult, accum_out=dinc[:])
                    nc.vector.tensor_add(out=den[:], in0=den[:], in1=dinc[:])
                    hw_ = ap_.tile([128, CH, 256], BF16, tag="hw",
                                   name=f"hw{sfx}_{bi}")
                    nc.sync.dma_start(out=hw_[:], in_=hist_d.ap()[cf])
                    for u in range(CH):
                        nc.vector.scalar_tensor_tensor(
                            out=acc_v[:], in0=hw_[:, u], scalar=e_blk[:, u:u + 1],
                            in1=acc_v[:], op0=OP.mult, op1=OP.add)

            # ---------------- tick loop ----------------
            for tick in range(NT):
                if tick < NCH:
                    with tc.If(role == 0, name=f"L0t{tick}"):
                        emit_lstm_chunk(tick, False, f"a{tick}")

                j1 = tick - LAG
                w = tick - WSTART

                # slot-asymmetric DMA: pull the peer's a-chunk(s) for window w
                # into the aremd DRAM mirror (pid1 reads slot1, pid3 slot0)
                if 0 <= w < NW:
                    for slot, cpid in ((1, 1), (0, 3)):
                        with tc.If(pid == cpid, name=f"AR{tick}_{cpid}"):
                            for bi, bs in enumerate((MID - CH * (w + 1),
                                                     MID + CH * w)):
                                c_rem = NCH - 1 - bs // CH
                                nc.sync.dma_start(
                                    out=aremd[w % 2].ap()[bi],
                                    in_=agbuf2[c_rem].ap()
                                    [slot * 128:(slot + 1) * 128])

                do_l1 = 0 <= j1 < NCH
                do_w = 0 <= w < NW
                if do_l1 or do_w:
                    with tc.If(role == 1, name=f"L1t{tick}"):
                        if do_l1:
                            emit_lstm_chunk(j1, True, f"b{tick}")
                        if do_w:
                            emit_window(w, f"w{tick}")

                if tick == NT - 1:
                    with tc.If(role == 1, name="fin"):
                        rden = ap_.tile([128, 1], F32, tag="dinc", name="rden")
                        nc.vector.reciprocal(out=rden[:], in_=den[:])
                        yt = ap_.tile([128, 256], F32, tag="yt", name="yt")
                        nc.vector.tensor_scalar_mul(yt[:], acc_v[:], rden[:, 0:1])
                        nc.sync.dma_start(out=yout_d.ap(), in_=yt[:])

                # ---------------- collectives ----------------
                if tick < NCH:
                    nc.gpsimd.collective_compute(
                        "AllGather", mybir.AluOpType.bypass,
                        replica_groups=G1,
                        ins=[agin1[tick].ap()], outs=[agbuf1[tick].ap()])
                if 0 <= j1 < NCH:
                    nc.gpsimd.collective_compute(
                        "AllGather", mybir.AluOpType.bypass,
                        replica_groups=G2,
                        ins=[agin2[j1].ap()], outs=[agbuf2[j1].ap()])

    nc.compile()
    return nc


def _prep_lstm_w(Wih, Whh, bih, bhh, bf16):
    # torch gate order i,f,g,o -> device order g,o,i,f  (bank0=[g|o], bank1=[i|f])
    def reorder(M):
        return np.concatenate([M[512:768], M[768:1024], M[0:256], M[256:512]],
                              axis=0)

    wih_t = np.ascontiguousarray(reorder(np.asarray(Wih, np.float32)).T)
    whh_t = np.ascontiguousarray(reorder(np.asarray(Whh, np.float32)).T)
    bias = reorder((np.asarray(bih, np.float32)
                    + np.asarray(bhh, np.float32)).reshape(1024, 1)).reshape(1, 1024)

    def chunk(WT):
        return np.ascontiguousarray(
            WT.reshape(2, 128, 2, 512).transpose(1, 0, 2, 3)).astype(bf16)

    return chunk(wih_t), chunk(whh_t), bias.astype(bf16)


def kernel(**inputs):
    import ml_dtypes
    from concourse.bass_utils import run_bass_kernel_spmd

    bf16 = ml_dtypes.bfloat16
    x = np.asarray(inputs["x"], np.float32)
    Bv, S, Dv = x.shape
    if (S, "nc") not in _BUILD_CACHE:
        _BUILD_CACHE[(S, "nc")] = _build(S)
    nc = _BUILD_CACHE[(S, "nc")]

    xs_f = np.ascontiguousarray(x.transpose(1, 2, 0)).reshape(
        S, 2, 128, Bv).astype(bf16)
    xs_b = np.ascontiguousarray(x[:, ::-1].transpose(1, 2, 0)).reshape(
        S, 2, 128, Bv).astype(bf16)
    z_xs = np.zeros((S, 2, 128, Bv), bf16)
    zw = np.zeros((128, 2, 2, 512), bf16)
    zb = np.zeros((1, 1024), bf16)

    wf0 = _prep_lstm_w(inputs["Wih_f0"], inputs["Whh_f0"], inputs["bih_f0"],
                       inputs["bhh_f0"], bf16)
    wf1 = _prep_lstm_w(inputs["Wih_f1"], inputs["Whh_f1"], inputs["bih_f1"],
                       inputs["bhh_f1"], bf16)
    wb0 = _prep_lstm_w(inputs["Wih_b0"], inputs["Whh_b0"], inputs["bih_b0"],
                       inputs["bhh_b0"], bf16)
    wb1 = _prep_lstm_w(inputs["Wih_b1"], inputs["Whh_b1"], inputs["bih_b1"],
                       inputs["bhh_b1"], bf16)

    wa1 = np.asarray(inputs["Wa1"], np.float32)          # [A, 2H]
    wa2 = np.asarray(inputs["Wa2"], np.float32).reshape(1, A)

    def wa1half(cols):
        # [A, 256] -> [128, 2, A] (kc chunks of the 256 input dims)
        return np.ascontiguousarray(
            cols.T.reshape(2, 128, A).transpose(1, 0, 2)).astype(bf16)

    wa1f = wa1half(wa1[:, 0:H])
    wa1b = wa1half(wa1[:, H:2 * H])
    zwa1 = np.zeros((128, 2, A), bf16)
    wa2b = wa2.astype(bf16)
    mconst = np.float32(np.abs(wa2).sum())
    mneg = np.full((1, 1), -mconst, np.float32)

    def imap(xs, w3, wa1t):
        wih, whh, bias = w3
        return {"xs": xs, "wih": wih, "whh": whh, "bias": bias,
                "wa1t": wa1t, "wa2r": wa2b, "mneg": mneg}

    zero3 = (zw, zw, zb)
    in_maps = [
        imap(xs_f, wf0, zwa1), imap(z_xs, wf1, wa1f),
        imap(xs_b, wb0, zwa1), imap(z_xs, wb1, wa1b),
        imap(z_xs, zero3, zwa1), imap(z_xs, zero3, zwa1),
        imap(z_xs, zero3, zwa1), imap(z_xs, zero3, zwa1),
    ]
    global _last_in_maps
    _last_in_maps = in_maps
    res = run_bass_kernel_spmd(nc, in_maps, core_ids=list(range(8)))
    out = np.concatenate([res.results[1]["yout"], res.results[3]["yout"]], axis=1)
    return out.astype(np.float32)


# revision 21
# speedup vs baseline: 1.4586x; 1.1402x over previous
"""BiLSTM (2-layer, bidirectional) encoder + attention pooling on 8 Trainium2 cores.

Topology (one SPMD program, roles selected by partition id):
  pid 0: layer-0 forward LSTM  (full batch 128)          role 0
  pid 2: layer-0 backward LSTM (host-reversed x)         role 0
  pid 1: layer-1 forward LSTM + f-side attention + f-half output   role 1
  pid 3: layer-1 backward LSTM + b-side attention + b-half output  role 1
  pid 4-7: idle (participate in collectives with garbage pairs)

Communication per 32-step chunk (all bf16):
  AG1 groups [[0,1],[2,3],[4,5],[6,7]]: L0 hidden states (transposed layout)
      2MB wire -> L1 consumes with lag 2.
  AG2 groups [[1,3],[0,2],[4,6],[5,7]]: A=64 attention pre-projections
      0.5MB wire -> peer attention scores.

Gates are reordered host-side to [g|o|i|f] so bank0 of the PSUM gate tile
finishes first and tanh(g) can start early; tanh and sigmoid share one ACT
table set so there are no table reloads.  exp for the softmax uses the exact
identity exp(z) = sig(z)/(1-sig(z)) with a fixed shift M = sum|Wa2| so scores
can be consumed in any order; windows run middle-out as both directions'
scores become available.
"""
import numpy as np

B, D, H, A = 128, 256, 256, 64
CH = 32

_BUILD_CACHE = {}
_last_in_maps = None


def _build(S):
    import concourse.bass as bass
    from concourse import bacc
    import concourse.mybir as mybir
    from concourse.tile import TileContext
    from concourse.masks import make_identity

    F32 = mybir.dt.float32
    BF16 = mybir.dt.bfloat16
    AF = mybir.ActivationFunctionType
    OP = mybir.AluOpType

    NCH = S // CH
    LAG = 2                       # L1 consumes L0 chunk j at tick j+LAG
    NW = NCH // 2                 # middle-out window pairs
    WSTART = NCH // 2 + LAG + 2   # first window tick
    NT = WSTART + NW + 1          # + finalize tick
    MID = S // 2

    G1 = [[0, 1], [2, 3], [4, 5], [6, 7]]
    G2 = [[1, 3], [0, 2], [4, 6], [5, 7]]

    nc = bacc.Bacc("TRN2", target_bir_lowering=False, debug=False, num_devices=8)

    xs_d = nc.dram_tensor("xs", [S, 2, 128, B], BF16, kind="ExternalInput")
    wih_d = nc.dram_tensor("wih", [128, 2, 2, 512], BF16, kind="ExternalInput")
    whh_d = nc.dram_tensor("whh", [128, 2, 2, 512], BF16, kind="ExternalInput")
    bias_d = nc.dram_tensor("bias", [1, 1024], BF16, kind="ExternalInput")
    wa1_d = nc.dram_tensor("wa1t", [128, 2, A], BF16, kind="ExternalInput")
    wa2_d = nc.dram_tensor("wa2r", [1, A], BF16, kind="ExternalInput")
    mneg_d = nc.dram_tensor("mneg", [1, 1], F32, kind="ExternalInput")
    yout_d = nc.dram_tensor("yout", [B, H], F32, kind="ExternalOutput")

    agin1 = [nc.dram_tensor(f"agin1_{j}", [CH, 128, 256], BF16) for j in range(NCH)]
    agbuf1 = [nc.dram_tensor(f"agbuf1_{j}", [2 * CH, 128, 256], BF16)
              for j in range(NCH)]
    agin2 = [nc.dram_tensor(f"agin2_{j}", [128, CH, A], BF16) for j in range(NCH)]
    agbuf2 = [nc.dram_tensor(f"agbuf2_{j}", [2 * 128, CH, A], BF16)
              for j in range(NCH)]
    hist_d = nc.dram_tensor("hist", [NCH, 128, CH, 256], BF16)
    # per-pid slot pull lands here (ring of 2 window ticks, 2 blocks each)
    aremd = [nc.dram_tensor(f"aremd_{k}", [2, 128, CH, A], BF16) for k in range(2)]

    with TileContext(nc) as tc:
        with tc.tile_pool(name="wpool", bufs=1) as wp, \
             tc.tile_pool(name="xpool", bufs=2) as xp, \
             tc.tile_pool(name="gpool", bufs=2) as gp, \
             tc.tile_pool(name="apool", bufs=2) as ap_, \
             tc.tile_pool(name="psum", bufs=2, space="PSUM") as pp:

            pid = nc.partition_id()
            role = pid % 2 + (pid // 4) * 2

            # ---------------- prologue: constants and state ----------------
            wih = wp.tile([128, 2, 2, 512], BF16, tag="wih")
            nc.gpsimd.dma_start(out=wih[:], in_=wih_d.ap())
            whh = wp.tile([128, 2, 2, 512], BF16, tag="whh")
            nc.gpsimd.dma_start(out=whh[:], in_=whh_d.ap())
            bias_r = wp.tile([1, 1024], BF16, tag="bias_r")
            nc.gpsimd.dma_start(out=bias_r[:], in_=bias_d.ap())
            wa1 = wp.tile([128, 2, A], BF16, tag="wa1")
            nc.gpsimd.dma_start(out=wa1[:], in_=wa1_d.ap())
            wa2bc = wp.tile([128, A], BF16, tag="wa2bc")
            nc.sync.dma_start(out=wa2bc[:], in_=wa2_d.ap().partition_broadcast(128))
            wa2rep = wp.tile([128, CH, A], BF16, tag="wa2rep")
            for r in range(CH):
                nc.vector.tensor_copy(out=wa2rep[:, r], in_=wa2bc[:])
            mneg = wp.tile([128, 1], F32, tag="mneg")
            nc.sync.dma_start(out=mneg[:], in_=mneg_d.ap().partition_broadcast(128))
            ones_r = wp.tile([1, 128], BF16, tag="ones_r")
            nc.gpsimd.memset(ones_r[:], 1.0)
            ident = wp.tile([128, 128], F32, tag="ident")
            make_identity(nc, ident[:])
            identb = wp.tile([128, 128], BF16, tag="identb")
            nc.vector.tensor_copy(out=identb[:], in_=ident[:])

            zini = wp.tile([128, 256], F32, tag="zini")
            nc.gpsimd.memset(zini[:], 0.0)
            # persistent prev-step transposed h (per role; only one role runs
            # per core but tiles are statically allocated for both)
            hTp0 = wp.tile([128, 2, 128], BF16, tag="hTp0")
            nc.gpsimd.memset(hTp0[:], 0.0)
            hTp1 = wp.tile([128, 2, 128], BF16, tag="hTp1")
            nc.gpsimd.memset(hTp1[:], 0.0)
            hT_prev = {0: hTp0, 1: hTp1}
            c_s = wp.tile([128, 256], F32, tag="c_s")
            nc.vector.tensor_copy(out=c_s[:], in_=zini[:])

            # ping-pong input-chunk tiles (persistent; prefetched a tick ahead)
            xtA0 = wp.tile([128, CH, 2, 128], BF16, tag="xtA0")
            xtB0 = wp.tile([128, CH, 2, 128], BF16, tag="xtB0")
            xtA1 = wp.tile([128, CH, 2, 128], BF16, tag="xtA1")
            xtB1 = wp.tile([128, CH, 2, 128], BF16, tag="xtB1")
            xt_pp = {0: (xtA0, xtB0), 1: (xtA1, xtB1)}
            # role0's chunk-0 input is ready in DRAM at start: prefetch now
            nc.sync.dma_start(
                out=xtA0[:],
                in_=xs_d.ap()[0:CH].rearrange("c a p b -> p c a b"))

            def emit_xt_prefetch(cj, is_l1):
                xt = xt_pp[1 if is_l1 else 0][cj % 2]
                if is_l1:
                    nc.sync.dma_start(
                        out=xt[:],
                        in_=agbuf1[cj].ap()[0:CH]
                        .rearrange("c p (a b) -> p c a b", a=2))
                else:
                    nc.sync.dma_start(
                        out=xt[:],
                        in_=xs_d.ap()[cj * CH:(cj + 1) * CH]
                        .rearrange("c a p b -> p c a b"))

            # attention state
            acc_v = wp.tile([128, 256], F32, tag="acc_v")
            nc.vector.tensor_copy(out=acc_v[:], in_=zini[:])
            den = wp.tile([128, 1], F32, tag="den")
            nc.vector.tensor_copy(out=den[:], in_=zini[:, 0:1])

            def emit_lstm_chunk(cj, is_l1, sfx):
                """One CH-step chunk of LSTM; role0 ships hT, role1 also
                computes attention projections and h history.  Gate layout
                [2g|o|i|f]; all nonlinearities are Sigmoid (tanh z =
                2*sig(2z)-1 with the 2z folded into weights host-side)."""
                rk = 1 if is_l1 else 0
                xt = xt_pp[rk][cj % 2]
                if cj + 1 < NCH:
                    emit_xt_prefetch(cj + 1, is_l1)
                hTc = xp.tile([128, CH, 2, 128], BF16, tag="hTc", name=f"hTc{sfx}")
                if is_l1:
                    hc = xp.tile([128, CH, 256], BF16, tag="hc", name=f"hc{sfx}")
                    a_ch = ap_.tile([128, CH, A], BF16, bufs=1, tag="ach", name=f"ach{sfx}")

                pgb = [None, None]  # rotating python refs for gb tiles

                def emit_partA(t0):
                    gb = pp.tile([128, 1088], F32, tag="gb", name=f"gb{sfx}_{t0}")
                    # alternate banks so PSUM drains overlap the next fill;
                    # pair matmuls per stationary to halve weight loads
                    nc.tensor.matmul(gb[:, 0:512], ones_r[:], bias_r[:, 0:512],
                                     start=True, stop=False)
                    nc.tensor.matmul(gb[:, 512:1024], ones_r[:], bias_r[:, 512:1024],
                                     start=True, stop=False)
                    for kc in range(2):
                        nc.tensor.matmul(gb[:, 0:512], xt[:, t0, kc], wih[:, kc, 0],
                                         start=False, stop=False)
                        nc.tensor.matmul(gb[:, 512:1024], xt[:, t0, kc],
                                         wih[:, kc, 1], start=False, stop=False)
                    return gb

                def emit_partB(gb, t0):
                    if t0 == 0:
                        prevT = hT_prev[rk][:]
                    else:
                        prevT = hTc[:, t0 - 1]
                    do_aps = is_l1 and t0 >= 1
                    nc.tensor.matmul(gb[:, 0:512], prevT[:, 0], whh[:, 0, 0],
                                     start=False, stop=False)
                    nc.tensor.matmul(gb[:, 512:1024], prevT[:, 0], whh[:, 0, 1],
                                     start=False, stop=False)
                    if do_aps:
                        # attention projection of step t0-1 (same stationaries)
                        nc.tensor.matmul(gb[:, 1024:1088], prevT[:, 0], wa1[:, 0],
                                         start=True, stop=False)
                    nc.tensor.matmul(gb[:, 0:512], prevT[:, 1], whh[:, 1, 0],
                                     start=False, stop=True)
                    nc.tensor.matmul(gb[:, 512:1024], prevT[:, 1], whh[:, 1, 1],
                                     start=False, stop=True)
                    if do_aps:
                        nc.tensor.matmul(gb[:, 1024:1088], prevT[:, 1], wa1[:, 1],
                                         start=False, stop=True)
                    sg = gp.tile([128, 1024], F32, tag="sg", name=f"sg{sfx}_{t0}")
                    nc.scalar.activation(sg[:, 0:512], gb[:, 0:512], AF.Sigmoid)
                    nc.scalar.activation(sg[:, 512:1024], gb[:, 512:1024],
                                         AF.Sigmoid)
                    # layout: sig_2g | sig_o | sig_i | sig_f
                    tg = gp.tile([128, 256], F32, tag="tg", name=f"tg{sfx}_{t0}")
                    nc.gpsimd.tensor_scalar(out=tg[:], in0=sg[:, 0:256],
                                            scalar1=2.0, scalar2=-1.0,
                                            op0=OP.mult, op1=OP.add)
                    v = gp.tile([128, 256], F32, tag="v", name=f"v{sfx}_{t0}")
                    nc.gpsimd.tensor_mul(out=v[:], in0=tg[:], in1=sg[:, 512:768])
                    nc.vector.tensor_mul(out=c_s[:], in0=c_s[:], in1=sg[:, 768:1024])
                    nc.vector.tensor_add(out=c_s[:], in0=c_s[:], in1=v[:])
                    th = gp.tile([128, 256], F32, tag="th", name=f"th{sfx}_{t0}")
                    nc.scalar.activation(th[:], c_s[:], AF.Sigmoid, scale=2.0)
                    w_t = gp.tile([128, 256], F32, tag="w_t", name=f"w{sfx}_{t0}")
                    nc.vector.tensor_scalar(out=w_t[:], in0=th[:],
                                            scalar1=2.0, scalar2=-1.0,
                                            op0=OP.mult, op1=OP.add)
                    if is_l1:
                        hdst = hc[:, t0]
                    else:
                        h0 = gp.tile([128, 256], BF16, tag="h0", name=f"h0{sfx}_{t0}")
                        hdst = h0[:]
                    nc.vector.tensor_mul(out=hdst, in0=sg[:, 256:512], in1=w_t[:])
                    tp = pp.tile([128, 256], BF16, tag="tp", name=f"tp{sfx}_{t0}")
                    nc.tensor.transpose(tp[:, 0:128], hdst[:, 0:128], identb[:])
                    nc.tensor.transpose(tp[:, 128:256], hdst[:, 128:256], identb[:])
                    nc.vector.tensor_copy(
                        out=hTc[:, t0].rearrange("p a b -> p (a b)"), in_=tp[:])
                    if is_l1 and t0 >= 1:
                        nc.vector.tensor_copy(out=a_ch[:, t0 - 1],
                                              in_=gb[:, 1024:1088])

                pgb[0] = emit_partA(0)
                for t0 in range(CH):
                    if t0 + 1 < CH:
                        pgb[1] = emit_partA(t0 + 1)
                    emit_partB(pgb[0], t0)
                    pgb[0] = pgb[1]

                if is_l1:
                    # tail: attention projection of step CH-1
                    aps_t = pp.tile([128, 64], F32, tag="tp", name=f"apst{sfx}")
                    nc.tensor.matmul(aps_t[:], hTc[:, CH - 1, 0], wa1[:, 0],
                                     start=True, stop=False)
                    nc.tensor.matmul(aps_t[:], hTc[:, CH - 1, 1], wa1[:, 1],
                                     start=False, stop=True)
                    nc.vector.tensor_copy(out=a_ch[:, CH - 1], in_=aps_t[:])
                    nc.sync.dma_start(out=agin2[cj].ap(), in_=a_ch[:])
                    nc.sync.dma_start(out=hist_d.ap()[cj], in_=hc[:])
                else:
                    nc.sync.dma_start(
                        out=agin1[cj].ap().rearrange("c p f -> p c f"),
                        in_=hTc[:].rearrange("p c a b -> p c (a b)"))
                nc.vector.tensor_copy(
                    out=hT_prev[1 if is_l1 else 0][:].rearrange("p a b -> p (a b)"),
                    in_=hTc[:, CH - 1].rearrange("p a b -> p (a b)"))

            def emit_window(w, sfx):
                """Score + weighted-accumulate for the window-pair w
                (own-time blocks around MID). Slot-independent part."""
                for bi, bs in enumerate((MID - CH * (w + 1), MID + CH * w)):
                    cf = bs // CH
                    aown = ap_.tile([128, CH, A], BF16, bufs=1, tag="aown",
                                    name=f"aown{sfx}_{bi}")
                    nc.sync.dma_start(out=aown[:], in_=agin2[cf].ap())
                    arem = ap_.tile([128, CH, A], BF16, bufs=1, tag="arem",
                                    name=f"arem{sfx}_{bi}")
                    nc.sync.dma_start(out=arem[:], in_=aremd[w % 2].ap()[bi])
                    asum = ap_.tile([128, CH, A], BF16, bufs=1, tag="asum",
                                    name=f"asum{sfx}_{bi}")
                    nc.vector.tensor_add(out=asum[:], in0=aown[:],
                                         in1=arem[:][:, ::-1, :])
                    # tanh via sigmoid: 2*sig(2z)-1; the 2z is folded into wa1,
                    # the 2* into wa2rep, and the -1 into the mneg shift
                    nc.scalar.activation(asum[:], asum[:], AF.Sigmoid)
                    nc.vector.tensor_mul(out=asum[:], in0=asum[:], in1=wa2rep[:])
                    sco = ap_.tile([128, CH], F32, tag="sco", name=f"sco{sfx}_{bi}")
                    nc.vector.tensor_reduce(out=sco[:], in_=asum[:],
                                            axis=mybir.AxisListType.X,
                                            op=OP.add)
                    nc.scalar.activation(sco[:], sco[:], AF.Sigmoid,
                                         bias=mneg[:, 0:1])
                    dtl = ap_.tile([128, CH], F32, tag="dtl", name=f"dtl{sfx}_{bi}")
                    nc.vector.tensor_scalar(out=dtl[:], in0=sco[:],
                                            scalar1=-1.0, scalar2=1.0,
                                            op0=OP.mult, op1=OP.add)
                    nc.vector.reciprocal(out=dtl[:], in_=dtl[:])
                    e_blk = ap_.tile([128, CH], F32, tag="e_blk",
                                     name=f"eb{sfx}_{bi}")
                    dinc = ap_.tile([128, 1], F32, tag="dinc",
                                    name=f"di{sfx}_{bi}")
                    nc.vector.scalar_tensor_tensor(
                        out=e_blk[:], in0=sco[:], scalar=1.0, in1=dtl[:],
                        op0=OP.mult, op1=OP.mult, accum_out=dinc[:])
                    nc.vector.tensor_add(out=den[:], in0=den[:], in1=dinc[:])
                    hw_ = ap_.tile([128, CH, 256], BF16, bufs=1, tag="hw",
                                   name=f"hw{sfx}_{bi}")
                    nc.sync.dma_start(out=hw_[:], in_=hist_d.ap()[cf])
                    for u in range(CH):
                        nc.vector.scalar_tensor_tensor(
                            out=acc_v[:], in0=hw_[:, u], scalar=e_blk[:, u:u + 1],
                            in1=acc_v[:], op0=OP.mult, op1=OP.add)

            # ---------------- tick loop ----------------
            for tick in range(NT):
                if tick < NCH:
                    with tc.If(role == 0, name=f"L0t{tick}"):
                        emit_lstm_chunk(tick, False, f"a{tick}")

                if tick == 1:
                    # role1's chunk-0 input became available via AG1[0]
                    with tc.If(role == 1, name="L1pf"):
                        emit_xt_prefetch(0, True)

                j1 = tick - LAG
                w = tick - WSTART

                # slot-asymmetric DMA: pull the peer's a-chunk(s) for window w
                # into the aremd DRAM mirror (pid1 reads slot1, pid3 slot0)
                if 0 <= w < NW:
                    for slot, cpid in ((1, 1), (0, 3)):
                        with tc.If(pid == cpid, name=f"AR{tick}_{cpid}"):
                            for bi, bs in enumerate((MID - CH * (w + 1),
                                                     MID + CH * w)):
                                c_rem = NCH - 1 - bs // CH
                                nc.sync.dma_start(
                                    out=aremd[w % 2].ap()[bi],
                                    in_=agbuf2[c_rem].ap()
                                    [slot * 128:(slot + 1) * 128])

                do_l1 = 0 <= j1 < NCH
                do_w = 0 <= w < NW
                if do_l1 or do_w:
                    with tc.If(role == 1, name=f"L1t{tick}"):
                        if do_l1:
                            emit_lstm_chunk(j1, True, f"b{tick}")
                        if do_w:
                            emit_window(w, f"w{tick}")

                if tick == NT - 1:
                    with tc.If(role == 1, name="fin"):
                        rden = ap_.tile([128, 1], F32, tag="dinc", name="rden")
                        nc.vector.reciprocal(out=rden[:], in_=den[:])
                        yt = ap_.tile([128, 256], F32, tag="yt", name="yt")
                        nc.vector.tensor_scalar_mul(yt[:], acc_v[:], rden[:, 0:1])
                        nc.sync.dma_start(out=yout_d.ap(), in_=yt[:])

                # ---------------- collectives ----------------
                if tick < NCH:
                    nc.gpsimd.collective_compute(
                        "AllGather", mybir.AluOpType.bypass,
                        replica_groups=G1,
                        ins=[agin1[tick].ap()], outs=[agbuf1[tick].ap()])
                if 0 <= j1 < NCH:
                    nc.gpsimd.collective_compute(
                        "AllGather", mybir.AluOpType.bypass,
                        replica_groups=G2,
                        ins=[agin2[j1].ap()], outs=[agbuf2[j1].ap()])

    nc.compile()
    return nc


def _prep_lstm_w(Wih, Whh, bih, bhh, bf16):
    # torch gate order i,f,g,o -> device order 2g,o,i,f (bank0=[2g|o],
    # bank1=[i|f]); the 2x on g feeds tanh(z) = 2*sig(2z)-1
    def reorder(M):
        return np.concatenate([2.0 * M[512:768], M[768:1024], M[0:256],
                               M[256:512]], axis=0)

    wih_t = np.ascontiguousarray(reorder(np.asarray(Wih, np.float32)).T)
    whh_t = np.ascontiguousarray(reorder(np.asarray(Whh, np.float32)).T)
    bias = reorder((np.asarray(bih, np.float32)
                    + np.asarray(bhh, np.float32)).reshape(1024, 1)).reshape(1, 1024)

    def chunk(WT):
        return np.ascontiguousarray(
            WT.reshape(2, 128, 2, 512).transpose(1, 0, 2, 3)).astype(bf16)

    return chunk(wih_t), chunk(whh_t), bias.astype(bf16)


def kernel(**inputs):
    import ml_dtypes
    from concourse.bass_utils import run_bass_kernel_spmd

    bf16 = ml_dtypes.bfloat16
    x = np.asarray(inputs["x"], np.float32)
    Bv, S, Dv = x.shape
    if (S, "nc") not in _BUILD_CACHE:
        _BUILD_CACHE[(S, "nc")] = _build(S)
    nc = _BUILD_CACHE[(S, "nc")]

    xs_f = np.ascontiguousarray(x.transpose(1, 2, 0)).reshape(
        S, 2, 128, Bv).astype(bf16)
    xs_b = np.ascontiguousarray(x[:, ::-1].transpose(1, 2, 0)).reshape(
        S, 2, 128, Bv).astype(bf16)
    z_xs = np.zeros((S, 2, 128, Bv), bf16)
    zw = np.zeros((128, 2, 2, 512), bf16)
    zb = np.zeros((1, 1024), bf16)

    wf0 = _prep_lstm_w(inputs["Wih_f0"], inputs["Whh_f0"], inputs["bih_f0"],
                       inputs["bhh_f0"], bf16)
    wf1 = _prep_lstm_w(inputs["Wih_f1"], inputs["Whh_f1"], inputs["bih_f1"],
                       inputs["bhh_f1"], bf16)
    wb0 = _prep_lstm_w(inputs["Wih_b0"], inputs["Whh_b0"], inputs["bih_b0"],
                       inputs["bhh_b0"], bf16)
    wb1 = _prep_lstm_w(inputs["Wih_b1"], inputs["Whh_b1"], inputs["bih_b1"],
                       inputs["bhh_b1"], bf16)

    wa1 = 2.0 * np.asarray(inputs["Wa1"], np.float32)    # [A, 2H]; 2z of tanh
    wa2 = np.asarray(inputs["Wa2"], np.float32).reshape(1, A)

    def wa1half(cols):
        # [A, 256] -> [128, 2, A] (kc chunks of the 256 input dims)
        return np.ascontiguousarray(
            cols.T.reshape(2, 128, A).transpose(1, 0, 2)).astype(bf16)

    wa1f = wa1half(wa1[:, 0:H])
    wa1b = wa1half(wa1[:, H:2 * H])
    zwa1 = np.zeros((128, 2, A), bf16)
    # score uses sum(2*wa2 * sig(2z)) with the constant -sum(wa2) absorbed
    # into the fixed softmax shift M
    wa2b = (2.0 * wa2).astype(bf16)
    mconst = np.float32(np.maximum(2.0 * wa2, 0.0).sum())
    mneg = np.full((1, 1), -mconst, np.float32)

    def imap(xs, w3, wa1t):
        wih, whh, bias = w3
        return {"xs": xs, "wih": wih, "whh": whh, "bias": bias,
                "wa1t": wa1t, "wa2r": wa2b, "mneg": mneg}

    zero3 = (zw, zw, zb)
    in_maps = [
        imap(xs_f, wf0, zwa1), imap(z_xs, wf1, wa1f),
        imap(xs_b, wb0, zwa1), imap(z_xs, wb1, wa1b),
        imap(z_xs, zero3, zwa1), imap(z_xs, zero3, zwa1),
        imap(z_xs, zero3, zwa1), imap(z_xs, zero3, zwa1),
    ]
    global _last_in_maps
    _last_in_maps = in_maps
    res = run_bass_kernel_spmd(nc, in_maps, core_ids=list(range(8)))
    out = np.concatenate([res.results[1]["yout"], res.results[3]["yout"]], axis=1)
    return out.astype(np.float32)
